# revision 1
# baseline (speedup 1.0000x reference)
"""Trainium2 Bass kernel for the tree-LSTM decoder (nn_Decoder).

Model (per batch item):
  T=256 sequential LSTM steps with a parent-state gather feeding the input,
  followed per step by two general-attention blocks and an output projection.

Strategy:
  - Data-parallel over batch: B=32 across 8 cores -> 4 items/core.
  - Phase A (batched): X1[t*4+b, :] = [emb_nt, emb_rule(prev), emb_rule(par), 1] @ [Wx^T; bias]
    precomputed for all timesteps (the embedding part of the LSTM gate input).
  - Phase B (sequential): per step only gates += [h_{t-1}; parent] @ [Whh; Wp]^T remains.
    Weights are the *moving* matmul operand (float32r, N=512 -> full PE rate); the
    tiny transposed state [128,4] is the stationary operand.  Parent states are
    gathered from an HBM history buffer via indirect DMA, issued one step ahead;
    the parent==t-1 case is handled on-chip via a host-provided flag mask.
  - Phase C (batched): both attentions + final tanh projection for all 256 steps
    per item, with queries/contexts kept feature-on-partition so only the
    softmax outputs and the final result need PE transposes.

All matmuls run as float32r (full fp32 storage; fast PE mode).
"""

import os
import numpy as np

import concourse.bass as bass
import concourse.bacc as bacc
import concourse.mybir as mybir
import concourse.tile as tile
from concourse.bass import IndirectOffsetOnAxis
from concourse.bass_utils import run_bass_kernel_spmd
from concourse.masks import make_identity

F32 = mybir.dt.float32
F32R = mybir.dt.float32r
I32 = mybir.dt.int32
AF = mybir.ActivationFunctionType
AX = mybir.AxisListType

B, H, E = 32, 512, 512
LS = LR = 512
G4 = 4 * H            # 2048
KE = 3 * E + 1        # 1537 (embeddings + ones column for bias)
NCORES = 8
BL = B // NCORES      # 4 local items

TT = int(os.environ.get("KERNEL_T_STEPS", "256"))
PHASES = os.environ.get("KERNEL_PHASES", "abc")
PROBE = os.environ.get("KERNEL_PROBE", "")
MT = TT * BL          # rows of X1 / xe (t-major, item-fast)
SENT = TT * BL        # sentinel row in h history buffer (zeros)

_BUILT = {}


def _r(x):
    return x


def _build(nc_cls=bacc.Bacc):
    nc = nc_cls("TRN2")

    # ---------------- I/O ----------------
    din = lambda n, s, d=F32: nc.dram_tensor(n, s, d, kind="ExternalInput")
    xeT = din("xeT", [KE, MT], F32R)                 # embeddings^T (+ones row), cols t*4+b
    x1w = din("x1w", [KE, G4], F32R)                 # [Wx^T; bih+bhh]
    w2s = din("w2s", [2 * H, G4], F32R)              # [Whh^T ; Wp^T]
    winsT = din("winsT", [H, H], F32R)
    woutsT = din("woutsT", [2 * H, H], F32R)
    winvT = din("winvT", [H, H], F32R)
    woutvT = din("woutvT", [2 * H, H], F32R)
    walT = din("walT", [3 * H, H], F32R)
    balr = din("balr", [128, 4])               # bal rearranged (hc p) -> p hc
    scT = din("scT", [BL, H, LS], F32R)              # src ctx^T per item [h, l]
    scN = din("scN", [BL, LS, H], F32R)              # src ctx per item [l, h]
    rcT = din("rcT", [BL, H, LR], F32R)
    rcN = din("rcN", [BL, LR, H], F32R)
    mka_s = din("mka_s", [BL, LS], F32R)             # (mask-1)*1e9 rows
    mka_r = din("mka_r", [BL, LR], F32R)
    gidxT = din("gidxT", [4 * BL, TT], I32)    # parent gather rows (hc,b) or SENT
    flagbT = din("flagbT", [BL, TT])           # 1.0 where parent == t-1 (unused; kept for layout)
    flagm = din("flagm", [TT, 128, BL])        # flag expanded [t, p, b]
    h0bp = din("h0bp", [BL, H])
    c0bp = din("c0bp", [BL, H])
    h0Tr = din("h0Tr", [128, 4, BL], F32R)     # h0^T rearranged (hc p) b -> p hc b
    c0Tr = din("c0Tr", [128, 4, BL])           # c0^T rearranged likewise
    onesr = din("onesr", [1, 128], F32R)
    id4r = din("id4r", [BL, BL], F32R)
    id16r = din("id16r", [16, 16], F32R)

    out_a = nc.dram_tensor("out_a", [BL, TT, H], F32, kind="ExternalOutput")
    out_sc = nc.dram_tensor("out_sc", [BL, TT, LR], F32, kind="ExternalOutput")

    with tile.TileContext(nc) as tc:
        with (
            tc.tile_pool(name="dram", bufs=1, space="DRAM") as dp,
            tc.tile_pool(name="const", bufs=1) as cp,
        ):
            hbuf = dp.tile([TT * 16 + 16, 128], F32R)   # h history rows (t,hc,b), + zero sentinel
            x1d = dp.tile([MT, G4], F32R)               # X1 rows t*4+b
            hbT = cp.tile([128, 4, TT + 1, BL], F32R)   # transposed h history (phases B+C)

            ident = cp.tile([128, 128], F32)
            make_identity(nc, ident)
            ones_row = cp.tile([1, 128], F32R)
            nc.sync.dma_start(ones_row, onesr[:])
            id4_sb = cp.tile([BL, BL], F32R)
            nc.sync.dma_start(id4_sb, id4r[:])
            id16_sb = cp.tile([16, 16], F32R)
            nc.sync.dma_start(id16_sb, id16r[:])
            zrow = cp.tile([128, H], F32)
            nc.vector.memset(zrow, 0.0)
            nrows = TT * 16 + 16
            for r0 in range(0, nrows, 128):
                nr = min(128, nrows - r0)
                nc.gpsimd.dma_start(hbuf[r0:r0 + nr, :], zrow[:nr, :128])

            gidx_sb = cp.tile([4 * BL, TT], I32)
            nc.sync.dma_start(gidx_sb, gidxT[:])
            flagm_sb = cp.tile([128, TT, BL], F32)
            for tq in range(0, TT, 64):
                nr = min(64, TT - tq)
                nc.sync.dma_start(
                    flagm_sb[:, tq:tq + nr, :],
                    flagm[tq:tq + nr].rearrange("t p b -> p t b"),
                )
            bal_sb = cp.tile([128, 4], F32)
            nc.sync.dma_start(bal_sb, balr[:])
            mks_sb = []
            mkr_sb = []
            for bl in range(BL):
                ts_ = cp.tile([1, LS], F32R, name=f"mks{bl}")
                nc.sync.dma_start(ts_, mka_s[bl:bl + 1, :])
                mks_sb.append(ts_)
                tr_ = cp.tile([1, LR], F32R, name=f"mkr{bl}")
                nc.sync.dma_start(tr_, mka_r[bl:bl + 1, :])
                mkr_sb.append(tr_)

            # w2 streams in during phase A (its pool outlives A, closes before C)
            pw2_ctx = tc.tile_pool(name="pb_w2", bufs=1)
            pw2 = pw2_ctx.__enter__()
            w2_sb = pw2.tile([128, 8, G4], F32R)
            for kc in range(8):
                nc.sync.dma_start(
                    w2_sb[:, kc, :], w2s[kc * 128:(kc + 1) * 128, :]
                )

            # ================= Phase A: X1 = xe @ [Wx^T; bias] =================
            MC = MT // 128
            with (
                tc.tile_pool(name="pa_xe", bufs=1) as pxe,
                tc.tile_pool(name="pa_w", bufs=15) as pw1,
                tc.tile_pool(name="pa_cp", bufs=4) as pcp,
                tc.tile_pool(name="pa_ps", bufs=4, space="PSUM") as pps,
            ):
                xeT_sb = pxe.tile([128, 12, MT], F32R)
                for kc in range(12):
                    nc.sync.dma_start(
                        xeT_sb[:, kc, :], xeT[kc * 128:(kc + 1) * 128, :]
                    )
                xeL_sb = pxe.tile([1, MT], F32R)
                nc.sync.dma_start(xeL_sb, xeT[1536:1537, :])
                for nb in (range(4) if 'a' in PHASES else []):
                    wts = []
                    for kc in range(12):
                        wt = pw1.tile([128, 512], F32R, tag="w1", bufs=13)
                        nc.sync.dma_start(
                            wt, x1w[kc * 128:(kc + 1) * 128, nb * 512:(nb + 1) * 512]
                        )
                        wts.append(wt)
                    wl = pw1.tile([1, 512], F32R, tag="w1l", bufs=2)
                    nc.sync.dma_start(wl, x1w[1536:1537, nb * 512:(nb + 1) * 512])
                    for mc in range(MC):
                        ps = pps.tile([128, 512], F32, tag="ps")
                        for kc in range(12):
                            nc.tensor.matmul(
                                ps,
                                lhsT=_r(xeT_sb[:, kc, mc * 128:(mc + 1) * 128]),
                                rhs=_r(wts[kc]),
                                start=(kc == 0), stop=False,
                            )
                        nc.tensor.matmul(
                            ps, lhsT=_r(xeL_sb[:, mc * 128:(mc + 1) * 128]),
                            rhs=_r(wl), start=False, stop=True,
                        )
                        ct = pcp.tile([128, 512], F32R, tag="cp")
                        nc.vector.tensor_copy(ct, ps)
                        nc.sync.dma_start(
                            x1d[mc * 128:(mc + 1) * 128, nb * 512:(nb + 1) * 512], ct
                        )

            # ================= Phase B: sequential LSTM =================
            with (
                tc.tile_pool(name="pb_x1", bufs=6) as px1,
                tc.tile_pool(name="pb_par", bufs=3) as ppar,
                tc.tile_pool(name="pb_state", bufs=2) as pst,
                tc.tile_pool(name="pb_pw", bufs=8) as ppw,
                tc.tile_pool(name="pb_gps", bufs=1, space="PSUM") as pgps,
                tc.tile_pool(name="pb_tps", bufs=4, space="PSUM") as ptps,
            ):
                nc.sync.dma_start(hbT[:, :, 0, :], h0Tr[:])
                c_T = pst.tile([128, 4, BL], F32, tag="c")

                def gather(t):
                    # 16 rows of 128 = parent h^T chunks for all (hc, b)
                    pr = ppar.tile([16, 128], F32R, tag="praw")
                    nc.gpsimd.indirect_dma_start(
                        out=pr, out_offset=None, in_=hbuf[:],
                        in_offset=IndirectOffsetOnAxis(ap=gidx_sb[:, t:t + 1], axis=0),
                    )
                    return pr

                nc.sync.dma_start(c_T, c0Tr[:])
                par_cur = gather(0)
                for t in (range(TT) if 'b' in PHASES else []):
                    # next gather issued before this step's h write (WAR keeps
                    # it off the critical path; sentinel+flag covers p==t-1)
                    par_nxt = gather(t + 1) if t + 1 < TT else None

                    # parent^T = T(gathered) + flag * h_{t-1}^T  (all [128,16])
                    psp = ptps.tile([128, 16], F32R, tag="tpsr", bufs=1)
                    nc.tensor.transpose(psp, par_cur, id16_sb)
                    tmp = ppw.tile([128, 4, BL], F32, tag="pwT")
                    nc.vector.tensor_mul(
                        tmp, hbT[:, :, t, :],
                        flagm_sb[:, t:t + 1, :].to_broadcast([128, 4, BL]),
                    )
                    parT = ppar.tile([128, 4, BL], F32R, tag="parT")
                    nc.vector.tensor_add(
                        parT, psp.rearrange("p (a b) -> p a b", a=4), tmp
                    )

                    x1t = px1.tile([BL, G4], F32R, tag="x1")
                    nc.sync.dma_start(x1t, x1d[t * BL:(t + 1) * BL, :])

                    gps = pgps.tile([BL, G4], F32, tag="gps")
                    for nb in (0, 1, 2, 3):       # i, f, g, o
                        col = slice(nb * 512, (nb + 1) * 512)
                        # X1 injected via identity(K=4) matmul opens the group
                        nc.tensor.matmul(
                            gps[:, col], lhsT=_r(id4_sb), rhs=_r(x1t[:, col]),
                            start=True, stop=False,
                        )
                        for kc in range(8):
                            lhs = hbT[:, kc, t, :] if kc < 4 else parT[:, kc - 4, :]
                            nc.tensor.matmul(
                                gps[:, col],
                                lhsT=_r(lhs),
                                rhs=_r(w2_sb[:, kc, col]),
                                start=False, stop=(kc == 7),
                            )

                        def act_T(func, colsl, base):
                            # activation in b-part from PSUM, then transpose
                            # to [128,(hc,b)] so products run 128-wide
                            a_bp = ppw.tile([BL, H], F32, tag="pw", bufs=4)
                            nc.scalar.activation(a_bp, gps[:, colsl], func)
                            aT = ptps.tile([128, 4, BL], F32, tag="tps", bufs=3)
                            for j in range(4):
                                nc.tensor.transpose(
                                    aT[:, j, :], a_bp[:, j * 128:(j + 1) * 128],
                                    ident[:BL, :BL],
                                )
                            return aT
                        if nb == 0:
                            siT_ps = act_T(AF.Sigmoid, slice(0, 512), 0)
                            siT = ppw.tile([128, 4, BL], F32, tag="pwT")
                            nc.vector.tensor_copy(siT, siT_ps)
                        elif nb == 1:
                            sfT = act_T(AF.Sigmoid, slice(512, 1024), 512)
                            t1 = ppw.tile([128, 4, BL], F32, tag="pwT")
                            nc.vector.tensor_mul(t1, sfT, c_T)
                        elif nb == 2:
                            tgT = act_T(AF.Tanh, slice(1024, 1536), 1024)
                            t2 = ppw.tile([128, 4, BL], F32, tag="pwT")
                            nc.vector.tensor_mul(t2, siT, tgT)
                            c_T = pst.tile([128, 4, BL], F32, tag="c")
                            nc.vector.tensor_add(c_T, t1, t2)
                            tcn = ppw.tile([128, 4, BL], F32, tag="pwT")
                            nc.scalar.activation(tcn, c_T, AF.Tanh)

                    soT = act_T(AF.Sigmoid, slice(1536, 2048), 1536)
                    # h^T lands directly in its history slot
                    nc.vector.tensor_mul(hbT[:, :, t + 1, :], soT, tcn)

                    # DRAM rows (t, hc, b) of 128 for future gathers
                    for hc in range(4):
                        r0 = t * 16 + hc * 4
                        nc.sync.dma_start(
                            hbuf[r0:r0 + BL, :].rearrange("b p -> p b"),
                            hbT[:, hc, t + 1, :],
                        )

                    par_cur = par_nxt

            pw2_ctx.__exit__(None, None, None)

            # ================= Phase C: attention + output =================
            NMT = TT // 128 if TT >= 128 else 1
            TC = TT // NMT                     # timestep chunk (<=128? no: 128)
            with (
                tc.tile_pool(name="pc_w", bufs=1) as pcw,
                tc.tile_pool(name="pc_ctx", bufs=2) as pctx,
                tc.tile_pool(name="pc_q", bufs=2) as pq,
                tc.tile_pool(name="pc_sm", bufs=3) as psm,
                tc.tile_pool(name="pc_ps", bufs=4, space="PSUM") as pcps,
                tc.tile_pool(name="pc_tp", bufs=4, space="PSUM") as pctp,
            ):
                def loadw(apT, kcs, name):
                    t_ = pcw.tile([128, kcs, H], F32R, tag=name)
                    for kc in range(kcs):
                        nc.sync.dma_start(
                            t_[:, kc, :], apT[kc * 128:(kc + 1) * 128, :]
                        )
                    return t_

                wins_sb = loadw(winsT, 4, "wins")
                wouts_sb = loadw(woutsT, 8, "wouts")
                winv_sb = loadw(winvT, 4, "winv")
                woutv_sb = loadw(woutvT, 8, "woutv")
                wal_sb = loadw(walT, 12, "wal")

                for bl in (range(BL) if 'c' in PHASES else []):
                    ctxTs = pctx.tile([128, 4, LS], F32R, tag="ctxTs")
                    for kc in range(4):
                        nc.sync.dma_start(
                            ctxTs[:, kc, :], scT[bl, kc * 128:(kc + 1) * 128, :]
                        )
                    ctxNs = pctx.tile([128, 4, H], F32R, tag="ctxNs")
                    for kc in range(4):
                        nc.sync.dma_start(
                            ctxNs[:, kc, :], scN[bl, kc * 128:(kc + 1) * 128, :]
                        )
                    ctxTr = pctx.tile([128, 4, LR], F32R, tag="ctxTr", bufs=1)
                    for kc in range(4):
                        nc.sync.dma_start(
                            ctxTr[:, kc, :], rcT[bl, kc * 128:(kc + 1) * 128, :]
                        )
                    ctxNr = pctx.tile([128, 4, H], F32R, tag="ctxNr", bufs=1)
                    for kc in range(4):
                        nc.sync.dma_start(
                            ctxNr[:, kc, :], rcN[bl, kc * 128:(kc + 1) * 128, :]
                        )

                    def hT_read(kc):
                        return hbT[:, kc, 1:TT + 1, bl]

                    def attn(q_read, win_sb, wout_sb, ctxT, ctxN, mk_sb, sc_out):
                        # qpT[h',t] = win^T.T @ qT
                        qpT = pq.tile([128, 4, TT], F32R, tag="qpT", bufs=1)
                        for mh in range(4):
                            ps = pcps.tile([128, TT], F32, tag="cps")
                            for kc in range(4):
                                nc.tensor.matmul(
                                    ps,
                                    lhsT=_r(win_sb[:, kc, mh * 128:(mh + 1) * 128]),
                                    rhs=_r(q_read(kc)),
                                    start=(kc == 0), stop=(kc == 3),
                                )
                            nc.vector.tensor_copy(qpT[:, mh, :], ps)
                        # scores[t,l] = qpT.T @ ctxT  (+ mask row via ones)
                        alignT = pq.tile([128, 4, TT], F32R, tag="alignT", bufs=1)
                        for mt in range(NMT):
                            ps = pcps.tile([128, LS], F32, tag="cps")
                            for kc in range(4):
                                nc.tensor.matmul(
                                    ps[:TC, :],
                                    lhsT=_r(qpT[:, kc, mt * TC:(mt + 1) * TC]),
                                    rhs=_r(ctxT[:, kc, :]),
                                    start=(kc == 0), stop=False,
                                )
                            nc.tensor.matmul(
                                ps[:TC, :], lhsT=_r(ones_row[:, :TC]),
                                rhs=_r(mk_sb[bl]),
                                start=False, stop=True,
                            )
                            # softmax over l (free dim)
                            nmx = psm.tile([128, 1], F32, tag="nmx")
                            nc.vector.reduce_max(nmx[:TC, :], ps[:TC, :], AX.X, negate=True)
                            esc = psm.tile([128, LS], F32, tag="esc")
                            rsm = psm.tile([128, 1], F32, tag="rsm")
                            nc.scalar.activation(
                                esc[:TC, :], ps[:TC, :], AF.Exp,
                                bias=nmx[:TC, :], accum_out=rsm[:TC, :],
                            )
                            rin = psm.tile([128, 1], F32, tag="rin")
                            nc.vector.reciprocal(rin[:TC, :], rsm[:TC, :])
                            alg = psm.tile([128, LS], F32, tag="alg")
                            nc.vector.tensor_scalar_mul(alg[:TC, :], esc[:TC, :], rin[:TC, :])
                            if sc_out is not None:
                                nc.sync.dma_start(
                                    sc_out[bl, mt * TC:(mt + 1) * TC, :], alg[:TC, :]
                                )
                            for lc in range(4):
                                tp = pctp.tile([128, 128], F32, tag="ctp")
                                nc.tensor.transpose(
                                    tp[:, :TC], alg[:TC, lc * 128:(lc + 1) * 128],
                                    ident[:TC, :TC],
                                )
                                nc.vector.tensor_copy(
                                    alignT[:, lc, mt * TC:(mt + 1) * TC], tp[:, :TC]
                                )
                        # cvecT[h,t] = ctxN.T @ alignT
                        cvT = pq.tile([128, 4, TT], F32R, tag="cvT", bufs=1)
                        for mh in range(4):
                            ps = pcps.tile([128, TT], F32, tag="cps")
                            for kc in range(4):
                                nc.tensor.matmul(
                                    ps,
                                    lhsT=_r(ctxN[:, kc, mh * 128:(mh + 1) * 128]),
                                    rhs=_r(alignT[:, kc, :]),
                                    start=(kc == 0), stop=(kc == 3),
                                )
                            nc.vector.tensor_copy(cvT[:, mh, :], ps)
                        # outT = tanh(wout^T.T @ [cvec; q])
                        oT = pq.tile([128, 4, TT], F32R, tag="oT")
                        for mh in range(4):
                            ps = pcps.tile([128, TT], F32, tag="cps")
                            for kc in range(8):
                                rhs = cvT[:, kc, :] if kc < 4 else q_read(kc - 4)
                                nc.tensor.matmul(
                                    ps,
                                    lhsT=_r(wout_sb[:, kc, mh * 128:(mh + 1) * 128]),
                                    rhs=_r(rhs),
                                    start=(kc == 0), stop=(kc == 7),
                                )
                            nc.scalar.activation(oT[:, mh, :], ps, AF.Tanh)
                        return oT

                    soT = attn(hT_read, wins_sb, wouts_sb, ctxTs, ctxNs, mks_sb, None)
                    voT = attn(
                        lambda kc: soT[:, kc, :], winv_sb, woutv_sb,
                        ctxTr, ctxNr, mkr_sb, out_sc,
                    )

                    # a^T = tanh(wal^T.T @ [h; so; vo] + bal)
                    aT = pq.tile([128, 4, TT], F32, tag="aT", bufs=1)
                    for mh in range(4):
                        ps = pcps.tile([128, TT], F32, tag="cps")
                        for kc in range(12):
                            if kc < 4:
                                rhs = hT_read(kc)
                            elif kc < 8:
                                rhs = soT[:, kc - 4, :]
                            else:
                                rhs = voT[:, kc - 8, :]
                            nc.tensor.matmul(
                                ps,
                                lhsT=_r(wal_sb[:, kc, mh * 128:(mh + 1) * 128]),
                                rhs=_r(rhs),
                                start=(kc == 0), stop=(kc == 11),
                            )
                        nc.scalar.activation(
                            aT[:, mh, :], ps, AF.Tanh, bias=bal_sb[:, mh:mh + 1]
                        )
                    # transpose a^T -> [t, h] and write out
                    for mt in range(NMT):
                        am = psm.tile([128, H], F32, tag="am")
                        for mh in range(4):
                            tp = pctp.tile([128, 128], F32, tag="ctp")
                            nc.tensor.transpose(
                                tp[:TC, :], aT[:, mh, mt * TC:(mt + 1) * TC], ident
                            )
                            nc.vector.tensor_copy(
                                am[:TC, mh * 128:(mh + 1) * 128], tp[:TC, :]
                            )
                        nc.sync.dma_start(out_a[bl, mt * TC:(mt + 1) * TC, :], am[:TC, :])

    nc.finalize()
    return nc


def _prep_core(inputs, c):
    s = slice(c * BL, (c + 1) * BL)
    f32 = lambda x: np.ascontiguousarray(np.asarray(x), dtype=np.float32)
    i64 = lambda x: np.asarray(x).astype(np.int64)

    nt = i64(inputs["nt"])[s, :TT]
    pr = i64(inputs["prev_rules"])[s, :TT]
    par = i64(inputs["parent_rules"])[s, :TT]
    pidx = i64(inputs["parent_idx"])[s, :TT]
    emb_nt = f32(inputs["emb_nt"])
    emb_rule = f32(inputs["emb_rule"])

    xe = np.concatenate(
        [emb_nt[nt], emb_rule[pr], emb_rule[par]], axis=-1
    )  # [BL, TT, 1536]
    xeT = np.empty((KE, TT * BL), np.float32)
    xeT[:1536] = xe.transpose(2, 1, 0).reshape(1536, TT * BL)
    xeT[1536] = 1.0

    tarr = np.arange(TT)[None, :]                      # [1, TT]
    flag = (pidx == tarr - 1).astype(np.float32)       # parent == t-1
    gath = pidx <= tarr - 2                            # [BL, TT]
    # rows of hbuf are (t, hc, b) 128-wide chunks; 16 indices per step
    hcv = np.arange(4)[:, None, None]                  # [4,1,1]
    bv = np.arange(BL)[None, :, None]                  # [1,BL,1]
    gidx = np.where(
        gath[None, :, :], pidx[None, :, :] * 16 + hcv * 4 + bv, TT * 16
    ).astype(np.int32).reshape(4 * BL, TT)             # row (hc*4+b)

    sc = f32(inputs["src_context"])[s]                 # [BL, LS, H]
    rc = f32(inputs["rest_context"])[s]
    smask = f32(inputs["src_mask"])[s]
    rmask = f32(inputs["rest_mask"])[s]
    h0 = f32(inputs["h0"])[s]
    c0 = f32(inputs["c0"])[s]
    h0T = h0.T                                          # [H, BL]

    return {
        "xeT": xeT,
        "gidxT": gidx,
        "flagbT": flag,
        "flagm": np.ascontiguousarray(
            np.broadcast_to(flag.T[:, None, :], (TT, 128, BL)).astype(np.float32)
        ),
        "scT": np.ascontiguousarray(sc.transpose(0, 2, 1)),
        "scN": sc,
        "rcT": np.ascontiguousarray(rc.transpose(0, 2, 1)),
        "rcN": rc,
        "mka_s": (smask - 1.0) * 1e9,
        "mka_r": (rmask - 1.0) * 1e9,
        "h0bp": h0,
        "c0bp": c0,
        "h0Tr": np.ascontiguousarray(h0T.reshape(4, 128, BL).transpose(1, 0, 2)),
        "c0Tr": np.ascontiguousarray(c0.T.reshape(4, 128, BL).transpose(1, 0, 2)),
    }


def _prep_shared(inputs):
    f32 = lambda x: np.ascontiguousarray(np.asarray(x), dtype=np.float32)
    Wih = f32(inputs["Wih"])        # [2048, 2048]
    Whh = f32(inputs["Whh"])        # [2048, 512]
    bih = f32(inputs["bih"])
    bhh = f32(inputs["bhh"])
    x1w = np.empty((KE, G4), np.float32)
    x1w[:1536] = Wih[:, :1536].T
    x1w[1536] = bih + bhh
    w2s = np.concatenate([Whh.T, Wih[:, 1536:2048].T], axis=0)  # [1024, 2048]
    bal = f32(inputs["bal"])
    return {
        "onesr": np.ones((1, 128), np.float32),
        "id4r": np.eye(BL, dtype=np.float32),
        "id16r": np.eye(16, dtype=np.float32),
        "x1w": x1w,
        "w2s": np.ascontiguousarray(w2s),
        "winsT": np.ascontiguousarray(f32(inputs["Win_s"]).T),
        "woutsT": np.ascontiguousarray(f32(inputs["Wout_s"]).T),
        "winvT": np.ascontiguousarray(f32(inputs["Win_v"]).T),
        "woutvT": np.ascontiguousarray(f32(inputs["Wout_v"]).T),
        "walT": np.ascontiguousarray(f32(inputs["Wal"]).T),
        "balr": np.ascontiguousarray(bal.reshape(4, 128).T),
    }


def kernel(**inputs):
    shared = _prep_shared(inputs)
    in_maps = []
    for c in range(NCORES):
        m = dict(shared)
        m.update(_prep_core(inputs, c))
        in_maps.append(m)

    if "nc" not in _BUILT:
        _BUILT["nc"] = _build()
    nc = _BUILT["nc"]

    trace = os.environ.get("KERNEL_TRACE", "0") == "1"
    res = run_bass_kernel_spmd(
        nc, in_maps, core_ids=list(range(NCORES)), trace=trace
    )
    _BUILT["last_result"] = res
    outs = res.results

    Tfull = np.asarray(inputs["nt"]).shape[1]
    output = np.zeros((B, Tfull, H), np.float32)
    scores = np.zeros((B, Tfull, LR), np.float32)
    for c in range(NCORES):
        output[c * BL:(c + 1) * BL, :TT] = outs[c]["out_a"]
        scores[c * BL:(c + 1) * BL, :TT] = outs[c]["out_sc"]
    return output, scores, scores



# revision 25
# speedup vs baseline: 2.9113x; 2.9113x over previous
"""Trainium2 Bass kernel for the tree-LSTM decoder (nn_Decoder).

Model (per batch item):
  T=256 sequential LSTM steps with a parent-state gather feeding the input,
  followed per step by two general-attention blocks and an output projection.

Strategy (v1 rewrite; cost-model-driven):
  - Data-parallel over batch: B=32 across 8 cores -> 4 items/core.
  - Phase A (batched, bf16): X1 = xe @ [Wx^T] + bias computed into SBUF
    (x1_sb), never round-tripping DRAM.
  - Phase B (sequential): gates are computed TRANSPOSED -- out chunks are
    [128 gate-rows, 4 batch] so each matmul streams only 4 columns (the
    PE cost model charges output-free-size per matmul; the stationary
    [128,128] weight load is free).  144 matmuls/step instead of
    36 x 512-wide streams: ~30x less PE time.
      * all four gate activations in ONE ACT op: tanh(0.5*gates) with the
        g-gate pre-scaled by 2 in the weights; sigma(x) = (tanh(x/2)+1)/2.
      * h is stored DOUBLED (2h); all weights consuming h are pre-halved.
      * c-chain uses fused scalar_tensor_tensor / tensor_tensor_reduce ops.
      * parent state: batched indirect DMA gather (2 steps per gather)
        from a DRAM history with DELAY=5 steps of lead; parents closer
        than the gather horizon are patched on-chip with copy_predicated
        and host-computed lag masks (lags 1..6).
  - Phase C (batched): attention in bf16, same structure as v0, with
    weights pre-scaled for the doubled-h convention.
"""

import os
import numpy as np
import ml_dtypes

import concourse.bass as bass
import concourse.bacc as bacc
import concourse.mybir as mybir
import concourse.tile as tile
from concourse.bass import IndirectOffsetOnAxis
from concourse.bass_utils import run_bass_kernel_spmd
from concourse.masks import make_identity

F32 = mybir.dt.float32
F32R = mybir.dt.float32r
BF16 = mybir.dt.bfloat16
I32 = mybir.dt.int32
U8 = mybir.dt.uint8
AF = mybir.ActivationFunctionType
AX = mybir.AxisListType
OP = mybir.AluOpType

B, H, E = 32, 512, 512
LS = LR = 512
G4 = 4 * H            # 2048
NCORES = 8
BL = B // NCORES      # 4 local items

TT = int(os.environ.get("KERNEL_T_STEPS", "256"))
PHASES = os.environ.get("KERNEL_PHASES", "abc")
MT = TT * BL          # rows of X1 (t-major, item-fast)
NCH = (MT + 127) // 128
SENT = TT * 16        # zero sentinel row in hbuf

DELAY = 5             # gather issued at step 2g serves steps 2g+5, 2g+6
LAGS = 6              # on-chip lag corrections 1..6
NO_GATHER = os.environ.get("KERNEL_NO_GATHER", "0") == "1"
NO_HWRITE = os.environ.get("KERNEL_NO_HWRITE", "0") == "1"
SIMPLE_VE = os.environ.get("KERNEL_SIMPLE_VE", "1") == "1"
NO_CP = os.environ.get("KERNEL_NO_CP", "0") == "1"
NG = 0 if NO_GATHER else max(0, (TT - DELAY + 1) // 2)  # number of gather ops

_BUILT = {}
bf16 = ml_dtypes.bfloat16


def _build(nc_cls=bacc.Bacc):
    nc = nc_cls("TRN2")

    din = lambda n, s, d: nc.dram_tensor(n, s, d, kind="ExternalInput")
    xeT = din("xeT", [1536, MT], BF16)          # embeddings^T, cols t*4+b
    onesm = din("onesm", [1, MT], F32R)
    x1wT = din("x1wT", [1536, G4], BF16)        # Wx^T (g-cols x2)
    x1wb = din("x1wb", [1, G4], F32R)           # bih+bhh (g-cols x2)
    w2s = din("w2s", [2 * H, G4], BF16)         # 0.5*[Whh^T; Wp^T], g-cols x2
    winsT = din("winsT", [H, H], BF16)          # 0.5 * Win_s^T
    woutsT = din("woutsT", [2 * H, H], BF16)    # Wout_s^T, q-rows x0.5
    winvT = din("winvT", [H, H], BF16)
    woutvT = din("woutvT", [2 * H, H], BF16)
    walT = din("walT", [3 * H, H], BF16)        # Wal^T, h-rows x0.5
    balr = din("balr", [128, 4], F32)
    scT = din("scT", [BL, H, LS], BF16)
    scN = din("scN", [BL, LS, H], BF16)
    rcT = din("rcT", [BL, H, LR], BF16)
    rcN = din("rcN", [BL, LR, H], BF16)
    mka_s = din("mka_s", [BL, LS], BF16)        # (mask-1)*1e9 rows
    mka_r = din("mka_r", [BL, LR], BF16)
    gidx2 = din("gidx2", [32, max(NG, 1)], I32) # gather rows, 2 steps/col
    maskl = din("maskl", [128, TT, LAGS, 16], U8)
    h0Tr = din("h0Tr", [128, 4, BL], BF16)      # (2*h0)^T rearranged
    c0Tr = din("c0Tr", [128, 4, BL], F32)
    id32r = din("id32r", [32, 32], BF16)
    id128r = din("id128r", [128, 128], BF16)
    onesr = din("onesr", [1, 128], BF16)

    out_a = nc.dram_tensor("out_a", [BL, TT, H], F32, kind="ExternalOutput")
    out_sc = nc.dram_tensor("out_sc", [BL, TT, LR], F32, kind="ExternalOutput")

    with tile.TileContext(nc) as tc:
        with (
            tc.tile_pool(name="dram", bufs=1, space="DRAM") as dp,
            tc.tile_pool(name="const", bufs=1) as cp,
        ):
            hbuf = dp.tile([TT * 16 + 16, 128], BF16)  # h rows (t, hc, b)

            x1T_sb = cp.tile([128, 16, MT], BF16)      # X1^T: gates on partition
            w2_sb = cp.tile([128, 8, G4], BF16)
            hbT = cp.tile([128, TT + 1, 4, BL], BF16)  # slot s = 2*h_{s-1}
            mask_sb = cp.tile([128, TT, LAGS, 16], U8)
            gidx_sb = cp.tile([32, max(NG, 1)], I32)
            id32b = cp.tile([32, 32], BF16)
            id128b = cp.tile([128, 128], BF16)
            ones_row = cp.tile([1, 128], BF16)
            bal_sb = cp.tile([128, 4], F32)
            zrow = cp.tile([1, 128], BF16)

            idf = cp.tile([128, 128], F32)
            make_identity(nc, idf)
            nc.sync.dma_start(id32b, id32r[:])
            nc.sync.dma_start(id128b, id128r[:])
            nc.sync.dma_start(ones_row, onesr[:])
            nc.sync.dma_start(bal_sb, balr[:])
            nc.sync.dma_start(gidx_sb, gidx2[:])
            for tq in range(0, TT, 64):
                nr = min(64, TT - tq)
                nc.sync.dma_start(mask_sb[:, tq:tq + nr], maskl[:, tq:tq + nr])
            for kc in range(8):
                nc.sync.dma_start(w2_sb[:, kc, :], w2s[kc * 128:(kc + 1) * 128, :])
            nc.vector.memset(zrow, 0.0)
            nc.sync.dma_start(hbuf[SENT:SENT + 1, :], zrow)
            nc.sync.dma_start(hbT[:, 0, :, :], h0Tr[:])

            mks_sb = []
            mkr_sb = []
            for bl in range(BL):
                ts_ = cp.tile([1, LS], BF16, name=f"mks{bl}")
                nc.sync.dma_start(ts_, mka_s[bl:bl + 1, :])
                mks_sb.append(ts_)
                tr_ = cp.tile([1, LR], BF16, name=f"mkr{bl}")
                nc.sync.dma_start(tr_, mka_r[bl:bl + 1, :])
                mkr_sb.append(tr_)

            # ======= Phase A: x1T_sb = (xe @ Wx^T + bias)^T (bf16, in SBUF) =======
            NRC = (MT + 511) // 512
            with (
                tc.tile_pool(name="pa_xe", bufs=1) as pxe,
                tc.tile_pool(name="pa_w", bufs=3) as pw1,
                tc.tile_pool(name="pa_ps", bufs=4, space="PSUM") as pps,
            ):
                xeT_sb = pxe.tile([128, 12, MT], BF16)
                for kc in range(12):
                    nc.sync.dma_start(
                        xeT_sb[:, kc, :], xeT[kc * 128:(kc + 1) * 128, :]
                    )
                onesm_sb = pxe.tile([1, MT], F32R)
                nc.sync.dma_start(onesm_sb, onesm[:])
                x1wb_sb = pxe.tile([1, G4], F32R)
                nc.sync.dma_start(x1wb_sb, x1wb[:])
                for mcg in (range(16) if 'a' in PHASES else []):
                    wt = pw1.tile([128, 12, 128], BF16, tag="w1")
                    for kc in range(12):
                        nc.sync.dma_start(
                            wt[:, kc, :],
                            x1wT[kc * 128:(kc + 1) * 128,
                                 mcg * 128:(mcg + 1) * 128],
                        )
                    for rc in range(NRC):
                        r = min(512, MT - rc * 512)
                        ps = pps.tile([128, 512], F32, tag="ps")
                        for kc in range(12):
                            nc.tensor.matmul(
                                ps[:, :r],
                                lhsT=wt[:, kc, :],
                                rhs=xeT_sb[:, kc, rc * 512:rc * 512 + r],
                                start=(kc == 0), stop=False,
                            )
                        nc.tensor.matmul(
                            ps[:, :r],
                            lhsT=x1wb_sb[:, mcg * 128:(mcg + 1) * 128],
                            rhs=onesm_sb[:, rc * 512:rc * 512 + r],
                            start=False, stop=True,
                        )
                        nc.vector.tensor_copy(
                            x1T_sb[:, mcg, rc * 512:rc * 512 + r], ps[:, :r]
                        )

            # ============ Phase B: sequential LSTM (transposed gates) ============
            with (
                tc.tile_pool(name="pb_par", bufs=5) as ppar,
                tc.tile_pool(name="pb_pr", bufs=4) as ppr,
                tc.tile_pool(name="pb_st", bufs=2) as pst,
                tc.tile_pool(name="pb_th", bufs=3) as pth,
                tc.tile_pool(name="pb_uv", bufs=4) as puv,
                tc.tile_pool(name="pb_gps", bufs=2, space="PSUM") as pgps,
                tc.tile_pool(name="pb_tps", bufs=2, space="PSUM") as ptps,
            ):
                c_T = pst.tile([128, 4, BL], F32, tag="c")
                nc.sync.dma_start(c_T, c0Tr[:])

                def horizon(t):
                    if NO_GATHER:
                        return -1
                    return 2 * ((t - DELAY) // 2) - 1 if t >= DELAY else -1

                tsps = {}

                def build_parT(tn, prs):
                    """Build parT for step tn: gather transpose + lags>=2."""
                    pT = ppar.tile([128, 4, BL], BF16, tag="parT")
                    if horizon(tn) >= 0:
                        g = (tn - DELAY) // 2
                        half = (tn - DELAY) % 2
                        if half == 0 or g not in tsps:
                            tsp = ptps.tile([128, 32], BF16, tag="tpsr", bufs=3)
                            nc.tensor.transpose(tsp, prs[g], id32b)
                            tsps[g] = tsp
                        nc.scalar.activation(
                            pT.rearrange("p a b -> p (a b)"),
                            tsps[g][:, half * 16:(half + 1) * 16], AF.Copy
                        )
                    else:
                        nc.vector.memset(pT, 0.0)
                    for l in range(2, LAGS + 1):
                        if tn - l >= 0 and not NO_CP:
                            nc.vector.copy_predicated(
                                pT.rearrange("p a b -> p (a b)"),
                                mask_sb[:, tn, l - 1, :],
                                hbT[:, tn - l + 1, :, :].rearrange(
                                    "p a b -> p (a b)"),
                            )
                    return pT

                prs = {}
                parts = {0: build_parT(0, prs)} if TT > 0 else {}
                for t in (range(TT) if 'b' in PHASES else []):
                    # issue batched gather serving steps t+5, t+6
                    if t % 2 == 0 and t // 2 < NG:
                        g = t // 2
                        pr = ppr.tile([32, 128], BF16, tag="praw")
                        nc.gpsimd.indirect_dma_start(
                            out=pr, out_offset=None, in_=hbuf[:],
                            in_offset=IndirectOffsetOnAxis(
                                ap=gidx_sb[:, g:g + 1], axis=0
                            ),
                        )
                        prs[g] = pr

                    pT = parts.pop(t)
                    if t >= 1 and not NO_CP:
                        # lag-1 patch: parent == h_{t-1} (slot t)
                        nc.vector.copy_predicated(
                            pT.rearrange("p a b -> p (a b)"),
                            mask_sb[:, t, 0, :],
                            hbT[:, t, :, :].rearrange("p a b -> p (a b)"),
                        )

                    # two PSUM banks, each ONE accumulation group spanning
                    # 8 gate chunks x 9 matmuls (zero region = whole bank)
                    g1 = pgps.tile([128, 8, BL], F32, tag="g80")
                    g2 = pgps.tile([128, 8, BL], F32, tag="g81")
                    gtiles = (g1, g2)
                    for half in range(2):
                        for mcl in range(8):
                            mc = half * 8 + mcl
                            nc.tensor.matmul(
                                gtiles[half][:, mcl, :],
                                lhsT=id128b,
                                rhs=x1T_sb[:, mc, t * BL:(t + 1) * BL],
                                start=(mcl == 0), stop=False,
                            )
                    for kc in range(4):
                        for half in range(2):
                            for mcl in range(8):
                                mc = half * 8 + mcl
                                nc.tensor.matmul(
                                    gtiles[half][:, mcl, :],
                                    lhsT=w2_sb[:, kc, mc * 128:(mc + 1) * 128],
                                    rhs=hbT[:, t, kc, :],
                                    start=False, stop=False,
                                )
                    for half in range(2):
                        for mcl in range(8):
                            mc = half * 8 + mcl
                            for kc in range(4, 8):
                                nc.tensor.matmul(
                                    gtiles[half][:, mcl, :],
                                    lhsT=w2_sb[:, kc, mc * 128:(mc + 1) * 128],
                                    rhs=pT[:, kc - 4, :],
                                    start=False,
                                    stop=(kc == 7 and mcl == 7),
                                )

                    # all activations as tanh: sigma(x) = (tanh(x/2)+1)/2
                    th = pth.tile([128, 16, BL], BF16, tag="th")
                    nc.scalar.activation(th[:, 0:8, :], g1, AF.Tanh, scale=0.5)
                    nc.scalar.activation(th[:, 8:16, :], g2, AF.Tanh, scale=0.5)
                    # c' = 0.5*((th_f+1)*c + (th_i+1)*th_g)
                    if SIMPLE_VE:
                        t1 = puv.tile([128, 4, BL], F32, tag="uv")
                        nc.vector.tensor_scalar_add(t1, th[:, 4:8, :], 1.0)
                        u = puv.tile([128, 4, BL], F32, tag="uv")
                        nc.vector.tensor_mul(u, t1, c_T)
                        t2 = puv.tile([128, 4, BL], F32, tag="uv")
                        nc.vector.tensor_scalar_add(t2, th[:, 0:4, :], 1.0)
                        v = puv.tile([128, 4, BL], F32, tag="uv")
                        nc.vector.tensor_mul(v, t2, th[:, 8:12, :])
                        cs = puv.tile([128, 4, BL], F32, tag="uv")
                        nc.vector.tensor_add(cs, u, v)
                        c_T = pst.tile([128, 4, BL], F32, tag="c")
                        nc.vector.tensor_scalar_mul(c_T, cs, 0.5)
                        thc = pth.tile([128, 4, BL], BF16, tag="thc", bufs=2)
                        nc.scalar.activation(thc, c_T, AF.Tanh)
                        t3 = puv.tile([128, 4, BL], F32, tag="uv")
                        nc.vector.tensor_scalar_add(t3, th[:, 12:16, :], 1.0)
                        nc.vector.tensor_mul(hbT[:, t + 1, :, :], t3, thc)
                    else:
                        u = puv.tile([128, 4, BL], F32, tag="uv")
                        nc.vector.scalar_tensor_tensor(
                            u, th[:, 4:8, :], 1.0, c_T, OP.add, OP.mult
                        )
                        v = puv.tile([128, 4, BL], F32, tag="uv")
                        nc.vector.scalar_tensor_tensor(
                            v, th[:, 0:4, :], 1.0, th[:, 8:12, :], OP.add, OP.mult
                        )
                        c_T = pst.tile([128, 4, BL], F32, tag="c")
                        dum = puv.tile([128, 1], F32, tag="dum", bufs=2)
                        nc.vector.tensor_tensor_reduce(
                            c_T, u, v, 0.5, 0.0, OP.add, OP.max, accum_out=dum
                        )
                        thc = pth.tile([128, 4, BL], BF16, tag="thc", bufs=2)
                        nc.scalar.activation(thc, c_T, AF.Tanh)
                        # 2h = (th_o+1)*tanh(c'), straight into the history slot
                        nc.vector.scalar_tensor_tensor(
                            hbT[:, t + 1, :, :], th[:, 12:16, :], 1.0, thc,
                            OP.add, OP.mult,
                        )

                    # history rows for future gathers (strided transpose DMA)
                    if not NO_HWRITE:
                        nc.sync.dma_start(
                            hbuf[t * 16:(t + 1) * 16, :].rearrange("r p -> p r"),
                            hbT[:, t + 1, :, :].rearrange("p a b -> p (a b)"),
                        )

                    if t + 1 < TT:
                        parts[t + 1] = build_parT(t + 1, prs)

            # ============ Phase C: attention + output (bf16) ============
            NMT = TT // 128 if TT >= 128 else 1
            TC = TT // NMT
            with (
                tc.tile_pool(name="pc_w", bufs=1) as pcw,
                tc.tile_pool(name="pc_ctx", bufs=2) as pctx,
                tc.tile_pool(name="pc_q", bufs=2) as pq,
                tc.tile_pool(name="pc_sm", bufs=3) as psm,
                tc.tile_pool(name="pc_ps", bufs=4, space="PSUM") as pcps,
                tc.tile_pool(name="pc_tp", bufs=2, space="PSUM") as pctp,
            ):
                def loadw(apT, kcs, name):
                    t_ = pcw.tile([128, kcs, H], BF16, tag=name)
                    for kc in range(kcs):
                        nc.sync.dma_start(
                            t_[:, kc, :], apT[kc * 128:(kc + 1) * 128, :]
                        )
                    return t_

                wins_sb = loadw(winsT, 4, "wins")
                wouts_sb = loadw(woutsT, 8, "wouts")
                winv_sb = loadw(winvT, 4, "winv")
                woutv_sb = loadw(woutvT, 8, "woutv")
                wal_sb = loadw(walT, 12, "wal")

                for bl in (range(BL) if 'c' in PHASES else []):
                    ctxTs = pctx.tile([128, 4, LS], BF16, tag="ctxTs")
                    for kc in range(4):
                        nc.sync.dma_start(
                            ctxTs[:, kc, :], scT[bl, kc * 128:(kc + 1) * 128, :]
                        )
                    ctxNs = pctx.tile([128, 4, H], BF16, tag="ctxNs")
                    for kc in range(4):
                        nc.sync.dma_start(
                            ctxNs[:, kc, :], scN[bl, kc * 128:(kc + 1) * 128, :]
                        )
                    ctxTr = pctx.tile([128, 4, LR], BF16, tag="ctxTr", bufs=1)
                    for kc in range(4):
                        nc.sync.dma_start(
                            ctxTr[:, kc, :], rcT[bl, kc * 128:(kc + 1) * 128, :]
                        )
                    ctxNr = pctx.tile([128, 4, H], BF16, tag="ctxNr", bufs=1)
                    for kc in range(4):
                        nc.sync.dma_start(
                            ctxNr[:, kc, :], rcN[bl, kc * 128:(kc + 1) * 128, :]
                        )

                    def hT_read(kc):
                        return hbT[:, 1:TT + 1, kc, bl]

                    def attn(q_read, win_sb, wout_sb, ctxT, ctxN, mk_sb, sc_out):
                        qpT = pq.tile([128, 4, TT], BF16, tag="qpT", bufs=1)
                        for mh in range(4):
                            ps = pcps.tile([128, TT], F32, tag="cps")
                            for kc in range(4):
                                nc.tensor.matmul(
                                    ps,
                                    lhsT=win_sb[:, kc, mh * 128:(mh + 1) * 128],
                                    rhs=q_read(kc),
                                    start=(kc == 0), stop=(kc == 3),
                                )
                            nc.vector.tensor_copy(qpT[:, mh, :], ps)
                        alignT = pq.tile([128, 4, TT], BF16, tag="alignT", bufs=1)
                        for mt in range(NMT):
                            ps = pcps.tile([128, LS], F32, tag="cps")
                            for kc in range(4):
                                nc.tensor.matmul(
                                    ps[:TC, :],
                                    lhsT=qpT[:, kc, mt * TC:(mt + 1) * TC],
                                    rhs=ctxT[:, kc, :],
                                    start=(kc == 0), stop=False,
                                )
                            nc.tensor.matmul(
                                ps[:TC, :], lhsT=ones_row[:, :TC],
                                rhs=mk_sb[bl],
                                start=False, stop=True,
                            )
                            nmx = psm.tile([128, 1], F32, tag="nmx")
                            nc.vector.reduce_max(nmx[:TC, :], ps[:TC, :], AX.X,
                                                 negate=True)
                            esc = psm.tile([128, LS], F32, tag="esc")
                            rsm = psm.tile([128, 1], F32, tag="rsm")
                            nc.scalar.activation(
                                esc[:TC, :], ps[:TC, :], AF.Exp,
                                bias=nmx[:TC, :], accum_out=rsm[:TC, :],
                            )
                            rin = psm.tile([128, 1], F32, tag="rin")
                            nc.vector.reciprocal(rin[:TC, :], rsm[:TC, :])
                            algb = psm.tile([128, LS], BF16, tag="algb")
                            nc.scalar.activation(
                                algb[:TC, :], esc[:TC, :], AF.Copy,
                                scale=rin[:TC, :],
                            )
                            if sc_out is not None:
                                algf = psm.tile([128, LS], F32, tag="algf")
                                nc.vector.tensor_scalar_mul(
                                    algf[:TC, :], esc[:TC, :], rin[:TC, :]
                                )
                                nc.sync.dma_start(
                                    sc_out[bl, mt * TC:(mt + 1) * TC, :],
                                    algf[:TC, :],
                                )
                            for lc in range(4):
                                tpc = pctp.tile([128, 128], BF16, tag="ctpb")
                                nc.tensor.transpose(
                                    tpc[:, :TC], algb[:TC, lc * 128:(lc + 1) * 128],
                                    id128b[:TC, :TC],
                                )
                                nc.vector.tensor_copy(
                                    alignT[:, lc, mt * TC:(mt + 1) * TC],
                                    tpc[:, :TC],
                                )
                        cvT = pq.tile([128, 4, TT], BF16, tag="cvT", bufs=1)
                        for mh in range(4):
                            ps = pcps.tile([128, TT], F32, tag="cps")
                            for kc in range(4):
                                nc.tensor.matmul(
                                    ps,
                                    lhsT=ctxN[:, kc, mh * 128:(mh + 1) * 128],
                                    rhs=alignT[:, kc, :],
                                    start=(kc == 0), stop=(kc == 3),
                                )
                            nc.vector.tensor_copy(cvT[:, mh, :], ps)
                        oT = pq.tile([128, 4, TT], BF16, tag="oT")
                        for mh in range(4):
                            ps = pcps.tile([128, TT], F32, tag="cps")
                            for kc in range(8):
                                rhs = cvT[:, kc, :] if kc < 4 else q_read(kc - 4)
                                nc.tensor.matmul(
                                    ps,
                                    lhsT=wout_sb[:, kc, mh * 128:(mh + 1) * 128],
                                    rhs=rhs,
                                    start=(kc == 0), stop=(kc == 7),
                                )
                            nc.scalar.activation(oT[:, mh, :], ps, AF.Tanh)
                        return oT

                    soT = attn(hT_read, wins_sb, wouts_sb, ctxTs, ctxNs,
                               mks_sb, None)
                    voT = attn(
                        lambda kc: soT[:, kc, :], winv_sb, woutv_sb,
                        ctxTr, ctxNr, mkr_sb, out_sc,
                    )

                    aT = pq.tile([128, 4, TT], F32, tag="aT", bufs=1)
                    for mh in range(4):
                        ps = pcps.tile([128, TT], F32, tag="cps")
                        for kc in range(12):
                            if kc < 4:
                                rhs = hT_read(kc)
                            elif kc < 8:
                                rhs = soT[:, kc - 4, :]
                            else:
                                rhs = voT[:, kc - 8, :]
                            nc.tensor.matmul(
                                ps,
                                lhsT=wal_sb[:, kc, mh * 128:(mh + 1) * 128],
                                rhs=rhs,
                                start=(kc == 0), stop=(kc == 11),
                            )
                        nc.scalar.activation(
                            aT[:, mh, :], ps, AF.Tanh, bias=bal_sb[:, mh:mh + 1]
                        )
                    for mt in range(NMT):
                        am = psm.tile([128, H], F32, tag="am")
                        for mh in range(4):
                            tpc = pctp.tile([128, 128], F32, tag="ctp")
                            nc.tensor.transpose(
                                tpc[:TC, :], aT[:, mh, mt * TC:(mt + 1) * TC],
                                idf,
                            )
                            nc.vector.tensor_copy(
                                am[:TC, mh * 128:(mh + 1) * 128], tpc[:TC, :]
                            )
                        nc.sync.dma_start(
                            out_a[bl, mt * TC:(mt + 1) * TC, :], am[:TC, :]
                        )

    nc.finalize()
    return nc


def _prep_shared(inputs):
    f32 = lambda x: np.ascontiguousarray(np.asarray(x), dtype=np.float32)
    b16 = lambda x: np.ascontiguousarray(np.asarray(x, dtype=np.float32)).astype(bf16)
    Wih = f32(inputs["Wih"])        # [2048, 2048]
    Whh = f32(inputs["Whh"])        # [2048, 512]
    bih = f32(inputs["bih"])
    bhh = f32(inputs["bhh"])

    x1wT = Wih[:, :1536].T.copy()   # [1536, 2048]
    x1wT[:, 1024:1536] *= 2.0       # g-gate x2 (tanh trick)
    x1wb = (bih + bhh).copy()[None, :]
    x1wb[:, 1024:1536] *= 2.0

    w2s = 0.5 * np.concatenate([Whh.T, Wih[:, 1536:2048].T], axis=0)
    w2s[:, 1024:1536] *= 2.0

    winsT = f32(inputs["Win_s"]).T * 0.5
    woutsT = f32(inputs["Wout_s"]).T.copy()
    woutsT[H:2 * H] *= 0.5
    walT = f32(inputs["Wal"]).T.copy()
    walT[0:H] *= 0.5
    bal = f32(inputs["bal"])

    return {
        "x1wT": b16(x1wT),
        "x1wb": np.ascontiguousarray(x1wb, dtype=np.float32),
        "w2s": b16(w2s),
        "winsT": b16(winsT),
        "woutsT": b16(woutsT),
        "winvT": b16(f32(inputs["Win_v"]).T),
        "woutvT": b16(f32(inputs["Wout_v"]).T),
        "walT": b16(walT),
        "balr": np.ascontiguousarray(bal.reshape(4, 128).T, dtype=np.float32),
        "id32r": np.eye(32, dtype=np.float32).astype(bf16),
        "id128r": np.eye(128, dtype=np.float32).astype(bf16),
        "onesr": np.ones((1, 128), np.float32).astype(bf16),
        "onesm": np.ones((1, MT), np.float32),
    }


def _prep_core(inputs, c):
    s = slice(c * BL, (c + 1) * BL)
    f32 = lambda x: np.ascontiguousarray(np.asarray(x), dtype=np.float32)
    b16 = lambda x: np.ascontiguousarray(np.asarray(x, dtype=np.float32)).astype(bf16)
    i64 = lambda x: np.asarray(x).astype(np.int64)

    nt = i64(inputs["nt"])[s, :TT]
    pr = i64(inputs["prev_rules"])[s, :TT]
    par = i64(inputs["parent_rules"])[s, :TT]
    pidx = i64(inputs["parent_idx"])[s, :TT]
    emb_nt = f32(inputs["emb_nt"])
    emb_rule = f32(inputs["emb_rule"])

    xe = np.concatenate(
        [emb_nt[nt], emb_rule[pr], emb_rule[par]], axis=-1
    )  # [BL, TT, 1536]
    xeT = np.ascontiguousarray(xe.transpose(2, 1, 0).reshape(1536, MT))

    # gather horizon per step: gather g issued at step 2g covers steps
    # 2g+DELAY, 2g+DELAY+1 and reads history rows <= 2g-1.
    def horizon(t):
        if NO_GATHER:
            return -1
        return 2 * ((t - DELAY) // 2) - 1 if t >= DELAY else -1

    hz = np.array([horizon(t) for t in range(TT)])          # [TT]
    tarr = np.arange(TT)[None, :]                           # [1, TT]

    # lag masks: parent == t-l and not covered by the gather
    maskl = np.zeros((TT, LAGS, BL), np.float32)
    for l in range(1, LAGS + 1):
        m = (pidx == tarr - l) & (pidx > hz[None, :]) & (tarr - l >= 0)
        maskl[:, l - 1, :] = m.T.astype(np.float32)
    maskl = np.tile(maskl, (1, 1, 4))                       # (hc,b) = 16 cols
    maskl = np.broadcast_to(maskl[None], (128, TT, LAGS, 16))

    # sanity: every parent < t is either gathered or lag-patched
    valid = pidx < tarr
    covered = (pidx <= hz[None, :]) | (tarr - pidx <= LAGS)
    assert NO_GATHER or np.all(~valid | covered), "gather/lag coverage hole"

    gidx2 = np.full((32, max(NG, 1)), SENT, np.int32)
    hcv = np.arange(4)[:, None]                             # [4, 1]
    bv = np.arange(BL)[None, :]                             # [1, BL]
    for g in range(NG):
        for dt_ in (0, 1):
            t = 2 * g + DELAY + dt_
            if t >= TT:
                continue
            p = pidx[:, t]                                  # [BL]
            ok = p <= horizon(t)                            # gatherable
            rows = np.where(
                ok[None, :], p[None, :] * 16 + hcv * 4 + bv, SENT
            )                                               # [4, BL]
            gidx2[dt_ * 16:(dt_ + 1) * 16, g] = rows.reshape(16)

    sc = f32(inputs["src_context"])[s]
    rc = f32(inputs["rest_context"])[s]
    smask = f32(inputs["src_mask"])[s]
    rmask = f32(inputs["rest_mask"])[s]
    h0 = f32(inputs["h0"])[s]
    c0 = f32(inputs["c0"])[s]

    return {
        "xeT": xeT.astype(bf16),
        "gidx2": gidx2,
        "maskl": np.ascontiguousarray(maskl).astype(np.uint8),
        "scT": b16(sc.transpose(0, 2, 1)),
        "scN": b16(sc),
        "rcT": b16(rc.transpose(0, 2, 1)),
        "rcN": b16(rc),
        "mka_s": b16((smask - 1.0) * 1e9),
        "mka_r": b16((rmask - 1.0) * 1e9),
        "h0Tr": b16((2.0 * h0).T.reshape(4, 128, BL).transpose(1, 0, 2)),
        "c0Tr": np.ascontiguousarray(
            c0.T.reshape(4, 128, BL).transpose(1, 0, 2), dtype=np.float32
        ),
    }


def kernel(**inputs):
    shared = _prep_shared(inputs)
    in_maps = []
    for c in range(NCORES):
        m = dict(shared)
        m.update(_prep_core(inputs, c))
        in_maps.append(m)

    if "nc" not in _BUILT:
        _BUILT["nc"] = _build()
    nc = _BUILT["nc"]

    trace = os.environ.get("KERNEL_TRACE", "0") == "1"
    res = run_bass_kernel_spmd(
        nc, in_maps, core_ids=list(range(NCORES)), trace=trace
    )
    _BUILT["last_result"] = res
    outs = res.results

    Tfull = np.asarray(inputs["nt"]).shape[1]
    output = np.zeros((B, Tfull, H), np.float32)
    scores = np.zeros((B, Tfull, LR), np.float32)
    for c in range(NCORES):
        output[c * BL:(c + 1) * BL, :TT] = outs[c]["out_a"]
        scores[c * BL:(c + 1) * BL, :TT] = outs[c]["out_sc"]
    return output, scores, scores


# revision 47
# speedup vs baseline: 4.0085x; 1.3769x over previous
"""Trainium2 Bass kernel for the tree-LSTM decoder (nn_Decoder).

Model (per batch item):
  T=256 sequential LSTM steps with a parent-state gather feeding the input,
  followed per step by two general-attention blocks and an output projection.

Strategy (v1 rewrite; cost-model-driven):
  - Data-parallel over batch: B=32 across 8 cores -> 4 items/core.
  - Phase A (batched, bf16): X1 = xe @ [Wx^T] + bias computed into SBUF
    (x1_sb), never round-tripping DRAM.
  - Phase B (sequential): gates are computed TRANSPOSED -- out chunks are
    [128 gate-rows, 4 batch] so each matmul streams only 4 columns (the
    PE cost model charges output-free-size per matmul; the stationary
    [128,128] weight load is free).  144 matmuls/step instead of
    36 x 512-wide streams: ~30x less PE time.
      * all four gate activations in ONE ACT op: tanh(0.5*gates) with the
        g-gate pre-scaled by 2 in the weights; sigma(x) = (tanh(x/2)+1)/2.
      * h is stored DOUBLED (2h); all weights consuming h are pre-halved.
      * c-chain as a short tree of plain DVE ops (the fused ant-dve ISA
        ops crash this runtime), one tanh ACT per gate bank + one for c.
      * parent state: per-step indirect DMA gather from a DRAM history
        with DELAY=4 steps of lead; parents closer than the horizon are
        patched on-chip with copy_predicated lag masks (lags 1..4).
      * phase-A matmuls and all parent-tile prep are emitted so they fill
        engine idle windows under the step's matmul phase.
  - Phase C (batched): attention in bf16, same structure as v0, with
    weights pre-scaled for the doubled-h convention.
"""

import os
import numpy as np
import ml_dtypes

import concourse.bass as bass
import concourse.bacc as bacc
import concourse.mybir as mybir
import concourse.tile as tile
from concourse.bass import IndirectOffsetOnAxis
from concourse.bass_utils import run_bass_kernel_spmd
from concourse.masks import make_identity

F32 = mybir.dt.float32
F32R = mybir.dt.float32r
BF16 = mybir.dt.bfloat16
I32 = mybir.dt.int32
U8 = mybir.dt.uint8
AF = mybir.ActivationFunctionType
AX = mybir.AxisListType
OP = mybir.AluOpType

B, H, E = 32, 512, 512
LS = LR = 512
G4 = 4 * H            # 2048
NCORES = 8
BL = B // NCORES      # 4 local items

TT = int(os.environ.get("KERNEL_T_STEPS", "256"))
PHASES = os.environ.get("KERNEL_PHASES", "abc")
MT = TT * BL          # rows of X1 (t-major, item-fast)
NCH = (MT + 127) // 128
SENT = 0              # zero sentinel row in hbuf (rows shifted +16)

DELAY = 4             # gather issued at step s serves step s+4
LAGS = 4              # on-chip lag corrections 1..4
NO_GATHER = os.environ.get("KERNEL_NO_GATHER", "0") == "1"
NO_HWRITE = os.environ.get("KERNEL_NO_HWRITE", "0") == "1"
NO_CP = os.environ.get("KERNEL_NO_CP", "0") == "1"

NG = 0 if NO_GATHER else max(0, TT - DELAY)  # number of gather ops

_BUILT = {}
bf16 = ml_dtypes.bfloat16


def _build(nc_cls=bacc.Bacc):
    nc = nc_cls("TRN2")

    din = lambda n, s, d: nc.dram_tensor(n, s, d, kind="ExternalInput")
    xeT = din("xeT", [1536, MT], BF16)          # embeddings^T, cols t*4+b
    onesm = din("onesm", [1, MT], F32R)
    x1wT = din("x1wT", [1536, G4], BF16)        # Wx^T (g-cols x2)
    x1wb = din("x1wb", [1, G4], F32R)           # bih+bhh (g-cols x2)
    w2s = din("w2s", [2 * H, G4], BF16)         # 0.5*[Whh^T; Wp^T], g-cols x2
    winsT = din("winsT", [H, H], BF16)          # 0.5 * Win_s^T
    woutsT = din("woutsT", [2 * H, H], BF16)    # Wout_s^T, q-rows x0.5
    winvT = din("winvT", [H, H], BF16)
    woutvT = din("woutvT", [2 * H, H], BF16)
    walT = din("walT", [3 * H, H], BF16)        # Wal^T, h-rows x0.5
    balr = din("balr", [128, 4], F32)
    scT = din("scT", [BL, H, LS], BF16)
    scN = din("scN", [BL, LS, H], BF16)
    rcT = din("rcT", [BL, H, LR], BF16)
    rcN = din("rcN", [BL, LR, H], BF16)
    mka_s = din("mka_s", [BL, LS], BF16)        # (mask-1)*1e9 rows
    mka_r = din("mka_r", [BL, LR], BF16)
    gidx2 = din("gidx2", [16, max(NG, 1)], I32) # gather rows, col g = step g+4
    maskl = din("maskl", [128, TT, LAGS, 4, BL], U8)
    h0Tr = din("h0Tr", [128, 4, BL], BF16)      # (2*h0)^T rearranged
    c0Tr = din("c0Tr", [128, 4, BL], F32)
    id16r = din("id16r", [16, 16], BF16)
    id128r = din("id128r", [128, 128], BF16)
    onesr = din("onesr", [1, 128], BF16)

    out_a = nc.dram_tensor("out_a", [BL, TT, H], F32, kind="ExternalOutput")
    out_sc = nc.dram_tensor("out_sc", [BL, TT, LR], F32, kind="ExternalOutput")

    with tile.TileContext(nc) as tc:
        with (
            tc.tile_pool(name="dram", bufs=1, space="DRAM") as dp,
            tc.tile_pool(name="const", bufs=1) as cp,
        ):
            hbuf = dp.tile([TT * 16 + 16, 128], BF16)  # rows 16+(t,hc,b); row 0 = zero

            NKC = (TT + 31) // 32
            # X1^T (gates on partition), one tile per 32-step chunk so the
            # dripped phase-A writes never alias the B-loop reads
            x1ch = [cp.tile([128, 16, min(128, MT - k * 128)], BF16,
                            name=f"x1c{k}") for k in range(NKC)]
            w2_sb = cp.tile([128, 8, G4], BF16)
            hbT = cp.tile([128, TT + 1, 4, BL], BF16)  # slot s = 2*h_{s-1}
            mask_sb = cp.tile([128, TT, LAGS, 4, BL], U8)
            gidx_sb = cp.tile([16, max(NG, 1)], I32)
            id16b = cp.tile([16, 16], BF16)
            id128b = cp.tile([128, 128], BF16)
            ones_row = cp.tile([1, 128], BF16)
            bal_sb = cp.tile([128, 4], F32)
            zrow = cp.tile([1, 128], BF16)

            idf = cp.tile([128, 128], F32)
            make_identity(nc, idf)
            nc.sync.dma_start(id16b, id16r[:])
            nc.sync.dma_start(id128b, id128r[:])
            nc.sync.dma_start(ones_row, onesr[:])
            nc.sync.dma_start(bal_sb, balr[:])
            nc.sync.dma_start(gidx_sb, gidx2[:])
            for tq in range(0, TT, 64):
                nr = min(64, TT - tq)
                nc.sync.dma_start(mask_sb[:, tq:tq + nr], maskl[:, tq:tq + nr])
            for kc in range(8):
                nc.sync.dma_start(w2_sb[:, kc, :], w2s[kc * 128:(kc + 1) * 128, :])
            nc.vector.memset(zrow, 0.0)
            nc.sync.dma_start(hbuf[0:1, :], zrow)
            nc.sync.dma_start(hbT[:, 0, :, :], h0Tr[:])

            mks_sb = []
            mkr_sb = []
            for bl in range(BL):
                ts_ = cp.tile([1, LS], BF16, name=f"mks{bl}")
                nc.sync.dma_start(ts_, mka_s[bl:bl + 1, :])
                mks_sb.append(ts_)
                tr_ = cp.tile([1, LR], BF16, name=f"mkr{bl}")
                nc.sync.dma_start(tr_, mka_r[bl:bl + 1, :])
                mkr_sb.append(tr_)

            # ======= Phase A (loads; matmuls drip into the B loop) =======
            pa_ctx = tc.tile_pool(name="pa_xe", bufs=1)
            pxe = pa_ctx.__enter__()
            xeT_sb = pxe.tile([128, 12, MT], BF16)
            for kc in range(12):
                nc.sync.dma_start(
                    xeT_sb[:, kc, :], xeT[kc * 128:(kc + 1) * 128, :]
                )
            onesm_sb = pxe.tile([1, MT], F32R)
            nc.sync.dma_start(onesm_sb, onesm[:])
            x1wb_sb = pxe.tile([1, G4], F32R)
            nc.sync.dma_start(x1wb_sb, x1wb[:])
            w1a = pxe.tile([128, 12, G4], BF16)
            for kc in range(12):
                nc.sync.dma_start(w1a[:, kc, :], x1wT[kc * 128:(kc + 1) * 128, :])

            # ============ Phase B: sequential LSTM (transposed gates) ============
            with (
                tc.tile_pool(name="pb_par", bufs=5) as ppar,
                tc.tile_pool(name="pb_pr", bufs=4) as ppr,
                tc.tile_pool(name="pb_st", bufs=2) as pst,
                tc.tile_pool(name="pb_th", bufs=3) as pth,
                tc.tile_pool(name="pb_uv", bufs=26) as puv,
                tc.tile_pool(name="pb_gps", bufs=2, space="PSUM") as pgps,
                tc.tile_pool(name="pb_tps", bufs=2, space="PSUM") as ptps,
                tc.tile_pool(name="pa_ps", bufs=3, space="PSUM") as pap,
            ):
                cT = pst.tile([128, 4, BL], F32, tag="c")
                nc.sync.dma_start(cT, c0Tr[:])

                NKC = len(x1ch)

                def a_ops():
                    """Yield phase-A micro-ops: (chunk k, emit_fn)."""
                    for k in range(NKC):
                        cw = min(128, MT - k * 128)
                        for mcg in range(16):
                            ps = [None]

                            def mk_mm(kc, mcg=mcg, k=k, cw=cw, ps=ps):
                                def emit():
                                    if kc == 0:
                                        ps[0] = pap.tile(
                                            [128, 128], F32, tag="aps",
                                            name=f"aps{k}_{mcg}")
                                    if kc < 12:
                                        nc.tensor.matmul(
                                            ps[0][:, :cw],
                                            lhsT=w1a[:, kc,
                                                     mcg * 128:(mcg + 1) * 128],
                                            rhs=xeT_sb[:, kc,
                                                       k * 128:k * 128 + cw],
                                            start=(kc == 0), stop=False,
                                        )
                                    elif kc == 12:
                                        nc.tensor.matmul(
                                            ps[0][:, :cw],
                                            lhsT=x1wb_sb[:,
                                                         mcg * 128:(mcg + 1) * 128],
                                            rhs=onesm_sb[:,
                                                         k * 128:k * 128 + cw],
                                            start=False, stop=True,
                                        )
                                    else:
                                        nc.scalar.activation(
                                            x1ch[k][:, mcg, :cw], ps[0][:, :cw],
                                            AF.Copy)
                                return emit

                            for kc in range(14):
                                yield k, mk_mm(kc)

                a_iter = iter(a_ops() if 'a' in PHASES else [])
                a_next = next(a_iter, None)

                def drip(upto_k, budget=-1):
                    nonlocal a_next
                    n = 0
                    while a_next is not None and a_next[0] <= upto_k:
                        if 0 <= budget <= n:
                            break
                        a_next[1]()
                        n += 1
                        a_next = next(a_iter, None)

                # chunk 0 must be ready before step 0
                drip(0)
                if 'b' not in PHASES:
                    drip(NKC)

                def horizon(t):
                    if NO_GATHER:
                        return -1
                    return t - DELAY - 1

                tsps = {}

                def parT_base(tn, prs):
                    """Base parent tile for step tn (gather transpose/memset)."""
                    pT = ppar.tile([128, 4, BL], BF16, tag="parT")
                    if horizon(tn) >= 0:
                        g = tn - DELAY
                        tsp = ptps.tile([128, 16], BF16, tag="tpsr", bufs=1)
                        nc.tensor.transpose(tsp, prs.pop(g), id16b)
                        nc.vector.tensor_copy(
                            pT.rearrange("p a b -> p (a b)"), tsp,
                        )
                    else:
                        nc.vector.memset(pT, 0.0)
                    return pT

                def parT_lags(tn, pT, ls):
                    for l in ls:
                        if 2 <= l <= LAGS and tn - l >= 0 and not NO_CP:
                            nc.vector.copy_predicated(
                                pT, mask_sb[:, tn, l - 1, :, :],
                                hbT[:, tn - l + 1, :, :],
                            )

                prs = {}
                parts = {0: parT_base(0, prs)} if TT > 0 else {}
                for t in (range(TT) if 'b' in PHASES else []):
                    # issue the gather serving step t+4 (reads rows <= t-1)
                    if t < NG:
                        pr = ppr.tile([16, 128], BF16, tag="praw", bufs=6)
                        nc.gpsimd.indirect_dma_start(
                            out=pr, out_offset=None,
                            in_=hbuf[:16 + 16 * t],
                            in_offset=IndirectOffsetOnAxis(
                                ap=gidx_sb[:, t:t + 1], axis=0
                            ),
                        )
                        prs[t] = pr

                    pT = parts.pop(t)
                    if t >= 1 and not NO_CP:
                        # lag-1 patch: parent == h_{t-1} (slot t)
                        nc.vector.copy_predicated(
                            pT, mask_sb[:, t, 0, :, :], hbT[:, t, :, :],
                        )
                    # next step's parent tile: base + lags 2..4 are all ready
                    # by now and fill DVE idle time under the matmul phase
                    if t + 1 < TT:
                        pTn = parts[t + 1] = parT_base(t + 1, prs)
                        parT_lags(t + 1, pTn, (2, 3, 4))
                    # phase-A drip first: runs in the PE idle window before
                    # h arrives, never inside the critical matmul block
                    drip(t // 32)
                    drip(t // 32 + 1, budget=7)
                    # gates in TWO PSUM groups (i,f,g | o): the ifg group
                    # closes early so the whole c-chain starts sooner.
                    # th layout: i 0:4, f 4:8, g 8:12, o 12:16
                    g1 = pgps.tile([128, 12, BL], F32, tag="gps1")
                    g2 = pgps.tile([128, 4, BL], F32, tag="gps2")
                    tc4 = (t % 32) * BL
                    for mc in range(16):
                        gx, mx = (g1, mc) if mc < 12 else (g2, mc - 12)
                        nc.tensor.matmul(
                            gx[:, mx, :],
                            lhsT=id128b,
                            rhs=x1ch[t // 32][:, mc, tc4:tc4 + BL],
                            start=(mx == 0), stop=False,
                        )
                    for gx, mlo, mhi in ((g1, 0, 12), (g2, 12, 16)):
                        for kc in range(4):
                            for mc in range(mlo, mhi):
                                nc.tensor.matmul(
                                    gx[:, mc - mlo, :],
                                    lhsT=w2_sb[:, kc, mc * 128:(mc + 1) * 128],
                                    rhs=hbT[:, t, kc, :],
                                    start=False, stop=False,
                                )
                        for mc in range(mlo, mhi):
                            for kc in range(4, 8):
                                nc.tensor.matmul(
                                    gx[:, mc - mlo, :],
                                    lhsT=w2_sb[:, kc, mc * 128:(mc + 1) * 128],
                                    rhs=pT[:, kc - 4, :],
                                    start=False,
                                    stop=(kc == 7 and mc == mhi - 1),
                                )

                    # all activations as tanh: sigma(x) = (tanh(x/2)+1)/2
                    th = pth.tile([128, 16, BL], BF16, tag="th")
                    nc.scalar.activation(th[:, 0:12, :], g1, AF.Tanh, scale=0.5)
                    nc.scalar.activation(th[:, 12:16, :], g2, AF.Tanh,
                                         scale=0.5)
                    # 2c' = th_f*c + c + th_i*th_g + th_g
                    m1 = puv.tile([128, 4, BL], F32, tag="uv")
                    nc.vector.tensor_mul(m1, th[:, 4:8, :], cT)
                    m2 = puv.tile([128, 4, BL], F32, tag="uv")
                    nc.vector.tensor_mul(m2, th[:, 0:4, :], th[:, 8:12, :])
                    s1 = puv.tile([128, 4, BL], F32, tag="uv")
                    nc.vector.tensor_add(s1, m1, cT)
                    s2 = puv.tile([128, 4, BL], F32, tag="uv")
                    nc.vector.tensor_add(s2, m2, th[:, 8:12, :])
                    cs = puv.tile([128, 4, BL], F32, tag="uv")
                    nc.vector.tensor_add(cs, s1, s2)
                    t3 = puv.tile([128, 4, BL], F32, tag="uv")
                    nc.vector.tensor_scalar_add(t3, th[:, 12:16, :], 1.0)
                    thc = pth.tile([128, 4, BL], BF16, tag="thc", bufs=2)
                    nc.scalar.activation(thc, cs, AF.Tanh, scale=0.5)
                    # 2h = (th_o+1)*tanh(c') -- one hop after thc
                    nc.vector.tensor_mul(hbT[:, t + 1, :, :], t3, thc)
                    # true c for the next step (off the critical path)
                    cT = pst.tile([128, 4, BL], F32, tag="c")
                    nc.vector.tensor_scalar_mul(cT, cs, 0.5)

                    # history rows for future gathers (strided transpose DMA)
                    if not NO_HWRITE:
                        nc.sync.dma_start(
                            hbuf[16 + t * 16:32 + t * 16, :].rearrange(
                                "r p -> p r"),
                            hbT[:, t + 1, :, :].rearrange("p a b -> p (a b)"),
                        )

            pa_ctx.__exit__(None, None, None)

            # ============ Phase C: attention + output (bf16) ============
            NMT = TT // 128 if TT >= 128 else 1
            TC = TT // NMT
            with (
                tc.tile_pool(name="pc_w", bufs=1) as pcw,
                tc.tile_pool(name="pc_ctx", bufs=2) as pctx,
                tc.tile_pool(name="pc_q", bufs=2) as pq,
                tc.tile_pool(name="pc_sm", bufs=3) as psm,
                tc.tile_pool(name="pc_ps", bufs=4, space="PSUM") as pcps,
                tc.tile_pool(name="pc_tp", bufs=2, space="PSUM") as pctp,
            ):
                def loadw(apT, kcs, name):
                    t_ = pcw.tile([128, kcs, H], BF16, tag=name)
                    for kc in range(kcs):
                        nc.sync.dma_start(
                            t_[:, kc, :], apT[kc * 128:(kc + 1) * 128, :]
                        )
                    return t_

                wins_sb = loadw(winsT, 4, "wins")
                wouts_sb = loadw(woutsT, 8, "wouts")
                winv_sb = loadw(winvT, 4, "winv")
                woutv_sb = loadw(woutvT, 8, "woutv")
                wal_sb = loadw(walT, 12, "wal")

                for bl in (range(BL) if 'c' in PHASES else []):
                    ctxTs = pctx.tile([128, 4, LS], BF16, tag="ctxTs")
                    for kc in range(4):
                        nc.sync.dma_start(
                            ctxTs[:, kc, :], scT[bl, kc * 128:(kc + 1) * 128, :]
                        )
                    ctxNs = pctx.tile([128, 4, H], BF16, tag="ctxNs")
                    for kc in range(4):
                        nc.sync.dma_start(
                            ctxNs[:, kc, :], scN[bl, kc * 128:(kc + 1) * 128, :]
                        )
                    ctxTr = pctx.tile([128, 4, LR], BF16, tag="ctxTr", bufs=1)
                    for kc in range(4):
                        nc.sync.dma_start(
                            ctxTr[:, kc, :], rcT[bl, kc * 128:(kc + 1) * 128, :]
                        )
                    ctxNr = pctx.tile([128, 4, H], BF16, tag="ctxNr", bufs=1)
                    for kc in range(4):
                        nc.sync.dma_start(
                            ctxNr[:, kc, :], rcN[bl, kc * 128:(kc + 1) * 128, :]
                        )

                    def hT_read(kc):
                        return hbT[:, 1:TT + 1, kc, bl]

                    def attn(q_read, win_sb, wout_sb, ctxT, ctxN, mk_sb, sc_out):
                        qpT = pq.tile([128, 4, TT], BF16, tag="qpT", bufs=1)
                        for mh in range(4):
                            ps = pcps.tile([128, TT], F32, tag="cps")
                            for kc in range(4):
                                nc.tensor.matmul(
                                    ps,
                                    lhsT=win_sb[:, kc, mh * 128:(mh + 1) * 128],
                                    rhs=q_read(kc),
                                    start=(kc == 0), stop=(kc == 3),
                                )
                            nc.vector.tensor_copy(qpT[:, mh, :], ps)
                        alignT = pq.tile([128, 4, TT], BF16, tag="alignT", bufs=1)
                        for mt in range(NMT):
                            ps = pcps.tile([128, LS], F32, tag="cps")
                            for kc in range(4):
                                nc.tensor.matmul(
                                    ps[:TC, :],
                                    lhsT=qpT[:, kc, mt * TC:(mt + 1) * TC],
                                    rhs=ctxT[:, kc, :],
                                    start=(kc == 0), stop=False,
                                )
                            nc.tensor.matmul(
                                ps[:TC, :], lhsT=ones_row[:, :TC],
                                rhs=mk_sb[bl],
                                start=False, stop=True,
                            )
                            nmx = psm.tile([128, 1], F32, tag="nmx")
                            nc.vector.reduce_max(nmx[:TC, :], ps[:TC, :], AX.X,
                                                 negate=True)
                            esc = psm.tile([128, LS], F32, tag="esc")
                            rsm = psm.tile([128, 1], F32, tag="rsm")
                            nc.scalar.activation(
                                esc[:TC, :], ps[:TC, :], AF.Exp,
                                bias=nmx[:TC, :], accum_out=rsm[:TC, :],
                            )
                            rin = psm.tile([128, 1], F32, tag="rin")
                            nc.vector.reciprocal(rin[:TC, :], rsm[:TC, :])
                            algb = psm.tile([128, LS], BF16, tag="algb")
                            nc.scalar.activation(
                                algb[:TC, :], esc[:TC, :], AF.Copy,
                                scale=rin[:TC, :],
                            )
                            if sc_out is not None:
                                algf = psm.tile([128, LS], F32, tag="algf")
                                nc.vector.tensor_scalar_mul(
                                    algf[:TC, :], esc[:TC, :], rin[:TC, :]
                                )
                                nc.sync.dma_start(
                                    sc_out[bl, mt * TC:(mt + 1) * TC, :],
                                    algf[:TC, :],
                                )
                            for lc in range(4):
                                tpc = pctp.tile([128, 128], BF16, tag="ctpb")
                                nc.tensor.transpose(
                                    tpc[:, :TC], algb[:TC, lc * 128:(lc + 1) * 128],
                                    id128b[:TC, :TC],
                                )
                                nc.vector.tensor_copy(
                                    alignT[:, lc, mt * TC:(mt + 1) * TC],
                                    tpc[:, :TC],
                                )
                        cvT = pq.tile([128, 4, TT], BF16, tag="cvT", bufs=1)
                        for mh in range(4):
                            ps = pcps.tile([128, TT], F32, tag="cps")
                            for kc in range(4):
                                nc.tensor.matmul(
                                    ps,
                                    lhsT=ctxN[:, kc, mh * 128:(mh + 1) * 128],
                                    rhs=alignT[:, kc, :],
                                    start=(kc == 0), stop=(kc == 3),
                                )
                            nc.vector.tensor_copy(cvT[:, mh, :], ps)
                        oT = pq.tile([128, 4, TT], BF16, tag="oT")
                        for mh in range(4):
                            ps = pcps.tile([128, TT], F32, tag="cps")
                            for kc in range(8):
                                rhs = cvT[:, kc, :] if kc < 4 else q_read(kc - 4)
                                nc.tensor.matmul(
                                    ps,
                                    lhsT=wout_sb[:, kc, mh * 128:(mh + 1) * 128],
                                    rhs=rhs,
                                    start=(kc == 0), stop=(kc == 7),
                                )
                            nc.scalar.activation(oT[:, mh, :], ps, AF.Tanh)
                        return oT

                    soT = attn(hT_read, wins_sb, wouts_sb, ctxTs, ctxNs,
                               mks_sb, None)
                    voT = attn(
                        lambda kc: soT[:, kc, :], winv_sb, woutv_sb,
                        ctxTr, ctxNr, mkr_sb, out_sc,
                    )

                    aT = pq.tile([128, 4, TT], F32, tag="aT", bufs=1)
                    for mh in range(4):
                        ps = pcps.tile([128, TT], F32, tag="cps")
                        for kc in range(12):
                            if kc < 4:
                                rhs = hT_read(kc)
                            elif kc < 8:
                                rhs = soT[:, kc - 4, :]
                            else:
                                rhs = voT[:, kc - 8, :]
                            nc.tensor.matmul(
                                ps,
                                lhsT=wal_sb[:, kc, mh * 128:(mh + 1) * 128],
                                rhs=rhs,
                                start=(kc == 0), stop=(kc == 11),
                            )
                        nc.scalar.activation(
                            aT[:, mh, :], ps, AF.Tanh, bias=bal_sb[:, mh:mh + 1]
                        )
                    for mt in range(NMT):
                        am = psm.tile([128, H], F32, tag="am")
                        for mh in range(4):
                            tpc = pctp.tile([128, 128], F32, tag="ctp")
                            nc.tensor.transpose(
                                tpc[:TC, :], aT[:, mh, mt * TC:(mt + 1) * TC],
                                idf,
                            )
                            nc.vector.tensor_copy(
                                am[:TC, mh * 128:(mh + 1) * 128], tpc[:TC, :]
                            )
                        nc.sync.dma_start(
                            out_a[bl, mt * TC:(mt + 1) * TC, :], am[:TC, :]
                        )

    nc.finalize()
    return nc


def _prep_shared(inputs):
    f32 = lambda x: np.ascontiguousarray(np.asarray(x), dtype=np.float32)
    b16 = lambda x: np.ascontiguousarray(np.asarray(x, dtype=np.float32)).astype(bf16)
    Wih = f32(inputs["Wih"])        # [2048, 2048]
    Whh = f32(inputs["Whh"])        # [2048, 512]
    bih = f32(inputs["bih"])
    bhh = f32(inputs["bhh"])

    x1wT = Wih[:, :1536].T.copy()   # [1536, 2048]
    x1wT[:, 1024:1536] *= 2.0       # g-gate x2 (tanh trick)
    x1wb = (bih + bhh).copy()[None, :]
    x1wb[:, 1024:1536] *= 2.0

    w2s = 0.5 * np.concatenate([Whh.T, Wih[:, 1536:2048].T], axis=0)
    w2s[:, 1024:1536] *= 2.0

    winsT = f32(inputs["Win_s"]).T * 0.5
    woutsT = f32(inputs["Wout_s"]).T.copy()
    woutsT[H:2 * H] *= 0.5
    walT = f32(inputs["Wal"]).T.copy()
    walT[0:H] *= 0.5
    bal = f32(inputs["bal"])

    return {
        "x1wT": b16(x1wT),
        "x1wb": np.ascontiguousarray(x1wb, dtype=np.float32),
        "w2s": b16(w2s),
        "winsT": b16(winsT),
        "woutsT": b16(woutsT),
        "winvT": b16(f32(inputs["Win_v"]).T),
        "woutvT": b16(f32(inputs["Wout_v"]).T),
        "walT": b16(walT),
        "balr": np.ascontiguousarray(bal.reshape(4, 128).T, dtype=np.float32),
        "id16r": np.eye(16, dtype=np.float32).astype(bf16),
        "id128r": np.eye(128, dtype=np.float32).astype(bf16),
        "onesr": np.ones((1, 128), np.float32).astype(bf16),
        "onesm": np.ones((1, MT), np.float32),
    }


def _prep_core(inputs, c):
    s = slice(c * BL, (c + 1) * BL)
    f32 = lambda x: np.ascontiguousarray(np.asarray(x), dtype=np.float32)
    b16 = lambda x: np.ascontiguousarray(np.asarray(x, dtype=np.float32)).astype(bf16)
    i64 = lambda x: np.asarray(x).astype(np.int64)

    nt = i64(inputs["nt"])[s, :TT]
    pr = i64(inputs["prev_rules"])[s, :TT]
    par = i64(inputs["parent_rules"])[s, :TT]
    pidx = i64(inputs["parent_idx"])[s, :TT]
    emb_nt = f32(inputs["emb_nt"])
    emb_rule = f32(inputs["emb_rule"])

    xe = np.concatenate(
        [emb_nt[nt], emb_rule[pr], emb_rule[par]], axis=-1
    )  # [BL, TT, 1536]
    xeT = np.ascontiguousarray(xe.transpose(2, 1, 0).reshape(1536, MT))

    # gather horizon per step: gather g issued at step 2g covers steps
    # 2g+DELAY, 2g+DELAY+1 and reads history rows <= 2g-1.
    def horizon(t):
        if NO_GATHER:
            return -1
        return t - DELAY - 1

    hz = np.array([horizon(t) for t in range(TT)])          # [TT]
    tarr = np.arange(TT)[None, :]                           # [1, TT]

    # lag masks: parent == t-l and not covered by the gather
    maskl = np.zeros((TT, LAGS, BL), np.float32)
    for l in range(1, LAGS + 1):
        m = (pidx == tarr - l) & (pidx > hz[None, :]) & (tarr - l >= 0)
        maskl[:, l - 1, :] = m.T.astype(np.float32)
    maskl = np.repeat(maskl[:, :, None, :], 4, axis=2)      # (hc, b)
    maskl = np.broadcast_to(maskl[None], (128, TT, LAGS, 4, BL))

    # sanity: every parent < t is either gathered or lag-patched
    valid = pidx < tarr
    covered = (pidx <= hz[None, :]) | (tarr - pidx <= LAGS)
    assert NO_GATHER or np.all(~valid | covered), "gather/lag coverage hole"

    gidx2 = np.full((16, max(NG, 1)), SENT, np.int32)
    hcv = np.arange(4)[:, None]                             # [4, 1]
    bv = np.arange(BL)[None, :]                             # [1, BL]
    for g in range(NG):
        t = g + DELAY
        p = pidx[:, t]                                      # [BL]
        ok = p <= horizon(t)                                # gatherable
        rows = np.where(
            ok[None, :], 16 + p[None, :] * 16 + hcv * 4 + bv, SENT
        )                                                   # [4, BL]
        gidx2[:, g] = rows.reshape(16)

    sc = f32(inputs["src_context"])[s]
    rc = f32(inputs["rest_context"])[s]
    smask = f32(inputs["src_mask"])[s]
    rmask = f32(inputs["rest_mask"])[s]
    h0 = f32(inputs["h0"])[s]
    c0 = f32(inputs["c0"])[s]

    return {
        "xeT": xeT.astype(bf16),
        "gidx2": gidx2,
        "maskl": np.ascontiguousarray(maskl).astype(np.uint8),
        "scT": b16(sc.transpose(0, 2, 1)),
        "scN": b16(sc),
        "rcT": b16(rc.transpose(0, 2, 1)),
        "rcN": b16(rc),
        "mka_s": b16((smask - 1.0) * 1e9),
        "mka_r": b16((rmask - 1.0) * 1e9),
        "h0Tr": b16((2.0 * h0).T.reshape(4, 128, BL).transpose(1, 0, 2)),
        "c0Tr": np.ascontiguousarray(
            c0.T.reshape(4, 128, BL).transpose(1, 0, 2), dtype=np.float32
        ),
    }


def kernel(**inputs):
    shared = _prep_shared(inputs)
    in_maps = []
    for c in range(NCORES):
        m = dict(shared)
        m.update(_prep_core(inputs, c))
        in_maps.append(m)

    if "nc" not in _BUILT:
        _BUILT["nc"] = _build()
    nc = _BUILT["nc"]

    trace = os.environ.get("KERNEL_TRACE", "0") == "1"
    res = run_bass_kernel_spmd(
        nc, in_maps, core_ids=list(range(NCORES)), trace=trace
    )
    _BUILT["last_result"] = res
    outs = res.results

    Tfull = np.asarray(inputs["nt"]).shape[1]
    output = np.zeros((B, Tfull, H), np.float32)
    scores = np.zeros((B, Tfull, LR), np.float32)
    for c in range(NCORES):
        output[c * BL:(c + 1) * BL, :TT] = outs[c]["out_a"]
        scores[c * BL:(c + 1) * BL, :TT] = outs[c]["out_sc"]
    return output, scores, scores
